# revision 39
# baseline (speedup 1.0000x reference)
"""Trainium2 Bass kernel for nn_BERTEmbedding (fused per-index affine + sinusoidal PE).

Math (per batch b, vocab-position v, embed index e):
    out[b,v,e] = s0[b,v]*flux_w[v,e] + flux_b[v,e]
               + s2[b,v]*time_w[v,e] + time_b[v,e]
               + (e even: sin(s1[b,v]*div[e/2]) ; e odd: cos(s1[b,v]*div[(e-1)/2]))

Sharding: vocab axis V=4096 split across 8 cores (512 rows each); every core
handles all 16 batches of its vocab shard.  Weight tables are sharded with the
vocab axis.

Device strategy (per core, 4 v-tiles x 16 batches = 64 work items of [128,768],
software-pipelined in 8 stages of GB=8 batches so every engine queue stays one
stage ahead of its consumers):
  - Tables fw/tw/bsum shipped bf16 (halves table DMA; bf16 matmuls are
    1 cycle/row at any moving width and halve LDWEIGHTS time).
  - TensorE: psum = diag(s0) @ fw + diag(s2) @ tw + I @ bsum, bf16 weights,
    f32 PSUM accumulate, 512/256 column splits per PSUM bank.
  - Diag tiles D[p, b*128+q] = eye[p,q] * s_ch[p,b]: batched GPSIMD broadcast
    chunks for vt0/vt2, per-item ScalarE Copy-with-scale for vt1/vt3 (plain 2D
    APs only - rearranged/squeezed scale APs corrupt ACT results).
  - ScalarE: pe tile via Sin activation per 8-batch group, laid out as
    [sin half | cos half].  ScalarE Sin valid on [-pi,pi]:
      k >= KLO: ang = s1*[dv_hi|dv2'] staged per-item on DVE (even v-tiles)
                or grouped GPSIMD broadcast mult (odd v-tiles);
                sin via bias=0, cos via bias=pi/2.
      k <  KLO: host ships integer phase codes combo_n (bf16-exact);
                r' = ang_lo + combo (one grouped GPSIMD add), then
                Sin(scale=pi/2).
  - VectorE: single merge out = psum + pe per item (interleaves sin/cos via
    read APs, doubles as the PSUM evacuation - DMA cannot read PSUM).
  - DMA: loads on the ACT HWDGE queue, stores on the SP queue (no
    head-of-line blocking between them); one 393KB store per work item.
NB (measured): GPSIMD execution stalls concurrent DVE ops, so GPSIMD work is
placed only where DVE has slack; perfetto slice durations include sem waits.
"""

import math

import numpy as np

try:
    import concourse.bass as bass
except ImportError:  # harness containers keep the repo at /opt/trn_rl_repo
    import sys

    sys.path.insert(0, "/opt/trn_rl_repo")
    import concourse.bass as bass

import concourse.bacc as bacc
import concourse.tile as tile
from concourse import mybir
from concourse.bass_utils import run_bass_kernel_spmd

B, V, E = 16, 4096, 768
EH = E // 2  # 384 angle lanes
KLO = 48  # angle lanes fixed up via the host combo tensor
KHI = EH - KLO  # 336 direct-sin lanes
# cos(ang_k) = 1.0 within ~1e-3 for k >= COSH (|ang| <= S1_LIMIT*10^(-k/96));
# those pe lanes are memset once instead of computed by ScalarE
COSH = 197
# sin(ang_k) = ang_k within ~1e-3 for k >= KSIN; staging writes ang into the
# pe tile directly and ScalarE only evaluates Sin on lanes [KLO, KSIN)
KSIN = 138
N_CORES = 8
V_SHARD = V // N_CORES  # 512
# merges per stage read psum directly on the DVE (1x); the rest go through a
# ScalarE psum->bf16 evacuation and an all-bf16 2x DVE add
N_DIRECT = 8
VT = V_SHARD // 128  # 4 v-tiles per core
GB = 8  # batches per pe/sin group
F32 = mybir.dt.float32
BF16 = mybir.dt.bfloat16

TWO_PI = 2.0 * math.pi
HALF_PI = float(np.float32(math.pi / 2.0))
# keep reduced angles strictly inside ScalarE's [-pi, pi] spline domain
SIN_SAFETY = 1.0 - 1e-6
# direct-Sin lanes need |s1|*d_KLO + pi/2 <= pi
S1_LIMIT = (math.pi / 2.0) / math.exp(-KLO * math.log(10000.0) / EH)



def build_bass() -> "bass.Bass":
    from contextlib import ExitStack

    nc = bacc.Bacc(
        "TRN2",
        target_bir_lowering=False,
        debug=False,
        num_devices=N_CORES,
    )
    Alu = mybir.AluOpType

    # dv_full = [ div[KLO:EH) (336) | dv2' (96, scaled 2/pi) ]
    KX = KHI + 2 * KLO  # 432
    seq_d = nc.dram_tensor("seq", [128, VT * B * 3], F32, kind="ExternalInput")
    fw_d = nc.dram_tensor("fw", [V_SHARD, E], BF16, kind="ExternalInput")
    tw_d = nc.dram_tensor("tw", [V_SHARD, E], BF16, kind="ExternalInput")
    bs_d = nc.dram_tensor("bs", [V_SHARD, E], BF16, kind="ExternalInput")
    dv_d = nc.dram_tensor("dv", [128, KX], F32, kind="ExternalInput")
    cmb_d = nc.dram_tensor("combo", [128, VT * B * 2 * KLO], BF16, kind="ExternalInput")
    eye_d = nc.dram_tensor("eye", [128, 128], BF16, kind="ExternalInput")
    # bf16 output, one [128, GB*E] block per pipeline stage; host reassembles
    NSTAGES = VT * (B // GB)
    out_d = nc.dram_tensor("out", [NSTAGES, 128, GB * E], BF16, kind="ExternalOutput")

    with tile.TileContext(nc) as tc, ExitStack() as ctx:
        const_pool = ctx.enter_context(tc.tile_pool(name="const", bufs=1))
        tab_pool = ctx.enter_context(tc.tile_pool(name="tables", bufs=2))
        dmat_pool = ctx.enter_context(tc.tile_pool(name="dmat", bufs=4))
        ang_pool = ctx.enter_context(tc.tile_pool(name="ang", bufs=4))
        ev_pool = ctx.enter_context(tc.tile_pool(name="ev", bufs=6))
        out_pool = ctx.enter_context(tc.tile_pool(name="out", bufs=4))
        psum_pool = ctx.enter_context(tc.tile_pool(name="psum", bufs=4, space="PSUM"))

        zero_t = const_pool.tile([128, 1], F32, tag="zero")
        nc.vector.memset(zero_t[:], 0.0)
        hpi_t = const_pool.tile([128, 1], F32, tag="hpi")
        nc.vector.memset(hpi_t[:], HALF_PI)

        seq_t = const_pool.tile([128, VT * B * 3], F32, tag="seq")
        nc.sync.dma_start(seq_t[:], seq_d[:])
        dv_t = const_pool.tile([128, KX], F32, tag="dv")
        nc.sync.dma_start(dv_t[:], dv_d[:])
        eye_t = const_pool.tile([128, 128], BF16, tag="eye")
        nc.sync.dma_start(eye_t[:], eye_d[:])

        # ping-pong pe tiles in the NATIVE interleaved layout (even col 2k =
        # sin lane k, odd col 2k+1 = cos lane k) so merges read flat APs.
        # cos lanes k >= COSH hold constant 1.0 (memset once, never
        # overwritten: ScalarE only writes sin + cos-head lanes)
        pe_tiles = []
        for pp in range(3):
            pe_t = const_pool.tile([128, GB * E], BF16, tag=f"pe{pp}")
            nc.vector.memset(
                pe_t[:].rearrange("p (i e) -> p i e", i=GB)[
                    :, :, 2 * COSH + 1 : E : 2
                ],
                1.0,
            )
            pe_tiles.append(pe_t)

        item_idx = 0
        # ---- software-pipelined emission over 8 stages (vt, g) ----
        # Stage k's production (angles, r4, sins, next-vtile diag builds) is
        # emitted BEFORE stage k-1's consumption (matmuls, merges, stores) so
        # no engine queue head-of-line blocks on a consumer stall.
        stages = [(vt, g) for vt in range(VT) for g in range(B // GB)]
        tabs: dict = {}
        dmats: dict = {}
        staged: dict = {}

        def seq_view(vt):
            return seq_t[:, vt * B * 3 : (vt + 1) * B * 3].rearrange(
                "p (b c) -> p b c", c=3
            )

        def load_tables(vt):
            fw_t = tab_pool.tile([128, E], BF16, tag="fw")
            nc.sync.dma_start(fw_t[:], fw_d[vt * 128 : (vt + 1) * 128, :])
            tw_t = tab_pool.tile([128, E], BF16, tag="tw")
            nc.sync.dma_start(tw_t[:], tw_d[vt * 128 : (vt + 1) * 128, :])
            bs_t = tab_pool.tile([128, E], BF16, tag="bs")
            nc.sync.dma_start(bs_t[:], bs_d[vt * 128 : (vt + 1) * 128, :])
            cmb_t = tab_pool.tile([128, B * 2 * KLO], BF16, tag="cmb")
            nc.sync.dma_start(
                cmb_t[:], cmb_d[:, vt * B * 2 * KLO : (vt + 1) * B * 2 * KLO]
            )
            tabs[vt] = (fw_t, tw_t, bs_t, cmb_t)

        def alloc_dmats(vt):
            dmats[vt] = {}
            for ch in (0, 2):
                d_t = dmat_pool.tile([128, B * 128], BF16, tag=f"d{ch}")
                dmats[vt][ch] = d_t

        def build_d(vt):
            # diag build D[p, b*128+q] = eye[p,q] * s_ch[p,b]:
            # batched broadcast chunks on GPSIMD for every v-tile
            sv = seq_view(vt)
            for ch in (0, 2):
                d3 = dmats[vt][ch][:].rearrange("p (b q) -> p b q", b=B)
                for h0 in (0, B // 2):
                    eye_b = eye_t[:].unsqueeze(1).broadcast_to((128, B // 2, 128))
                    s_b = sv[:, h0 : h0 + B // 2, ch : ch + 1].broadcast_to(
                        (128, B // 2, 128)
                    )
                    nc.gpsimd.tensor_tensor(
                        d3[:, h0 : h0 + B // 2, :], eye_b, s_b, Alu.mult
                    )

        def emit_stage(k, vt, g):
            g0, g1 = g * GB, (g + 1) * GB
            ang_g = ang_pool.tile([128, GB * 2 * KLO], F32, tag="ang")
            r4_g = ang_pool.tile([128, GB * 2 * KLO], F32, tag="r4")
            cmb_t = tabs[vt][3]
            pe_g = pe_tiles[k % 3]
            pe3 = pe_g[:].rearrange("p (i e) -> p i e", i=GB)
            sv = seq_view(vt)
            s1b = sv[:, g0:g1, 1:2]  # [128, GB, 1] positional channel
            # staging op2': ang_k for k >= KLO straight into the pe sin
            # lanes (sin(x)=x holds for k >= KSIN; lanes [KLO,KSIN) are
            # Sin-ed in place below; lanes [KSIN,COSH) feed cos first).
            # Per-item on ScalarE: the DVE queue (merges) is the pacer.
            for i, b in enumerate(range(g0, g1)):
                if i >= 6:
                    continue  # last two items staged as one grouped DVE op
                col = vt * B * 3 + b * 3 + 1
                nc.scalar.mul(
                    pe_g[:, i * E + 2 * KLO : (i + 1) * E : 2],
                    dv_t[:, 0:KHI],
                    seq_t[:, col : col + 1],
                )
            nc.vector.tensor_tensor(
                pe3[:, 6:GB, 2 * KLO : E : 2],
                dv_t[:, 0:KHI].unsqueeze(1).broadcast_to((128, 2, KHI)),
                s1b[:, 6:GB, :].broadcast_to((128, 2, KHI)),
                Alu.mult,
            )
            # grouped staging op1': lo r0 lanes (dv2' scaled 2/pi)
            nc.gpsimd.tensor_tensor(
                ang_g[:].rearrange("p (i l) -> p i l", i=GB),
                dv_t[:, KHI:KX].unsqueeze(1).broadcast_to((128, GB, 2 * KLO)),
                s1b.broadcast_to((128, GB, 2 * KLO)),
                Alu.mult,
            )
            # lo codes r' = r0 + combo, one grouped add
            nc.gpsimd.tensor_tensor(
                r4_g[:].rearrange("p (i l) -> p i l", i=GB),
                ang_g[:].rearrange("p (i l) -> p i l", i=GB),
                cmb_t[:, g * GB * 2 * KLO : (g + 1) * GB * 2 * KLO].rearrange(
                    "p (i l) -> p i l", i=GB
                ),
                Alu.add,
            )
            # cos head: reads the staged angles from the sin lanes (before
            # the in-place Sin consumes them), writes the odd cos columns
            nc.scalar.activation(
                pe3[:, :, 2 * KLO + 1 : 2 * COSH : 2],
                pe3[:, :, 2 * KLO : 2 * COSH : 2],
                mybir.ActivationFunctionType.Sin,
                bias=hpi_t[:],
                scale=1.0,
            )
            # sin head, in place over the staged angles
            nc.scalar.activation(
                pe3[:, :, 2 * KLO : 2 * KSIN : 2],
                pe3[:, :, 2 * KLO : 2 * KSIN : 2],
                mybir.ActivationFunctionType.Sin,
                bias=zero_t[:],
                scale=1.0,
            )
            # lo block: r4 layout per item is [48 sin | 48 cos]
            nc.scalar.activation(
                pe3[:, :, 0 : 2 * KLO].rearrange("p i (q h) -> p i h q", h=2),
                r4_g[:].rearrange("p (i h q) -> p i h q", i=GB, h=2),
                mybir.ActivationFunctionType.Sin,
                bias=zero_t[:],
                scale=HALF_PI * SIN_SAFETY,
            )
            return pe_g

        def emit_items(vt, g, pe_g):
            fw_t, tw_t, bs_t, _ = tabs[vt]
            st = vt * (B // GB) + g
            # per-stage bf16 output block, one batched DMA store per stage
            o_g = out_pool.tile([128, GB * E], BF16, tag="o")
            for i, b in enumerate(range(g * GB, (g + 1) * GB)):
                # psum = diag(s0)@fw + diag(s2)@tw + I@bsum, split 512/256
                # to keep each matmul inside one PSUM bank
                # stationary-major order so consecutive matmuls share one
                # LDWEIGHTS where walrus can elide the reload
                ps = psum_pool.tile([128, E], F32, tag="ps")
                for st_t, mov_t, first, last in (
                    (dmats[vt][0][:, b * 128 : (b + 1) * 128], fw_t, True, False),
                    (dmats[vt][2][:, b * 128 : (b + 1) * 128], tw_t, False, False),
                    (eye_t[:], bs_t, False, True),
                ):
                    for lo, hi in ((0, 512), (512, E)):
                        nc.tensor.matmul(
                            ps[:, lo:hi],
                            st_t,
                            mov_t[:, lo:hi],
                            start=first,
                            stop=last,
                        )
                # merge (flat APs: pe is natively interleaved).  Most items
                # go via a ScalarE psum->bf16 evacuation so the DVE add runs
                # in all-bf16 2x_1p mode; the rest read psum directly at 1x.
                if i < N_DIRECT:
                    nc.vector.tensor_tensor(
                        o_g[:, i * E : (i + 1) * E],
                        ps[:],
                        pe_g[:, i * E : (i + 1) * E],
                        Alu.add,
                    )
                else:
                    ev = ev_pool.tile([128, E], BF16, tag="ev")
                    nc.scalar.copy(ev[:], ps[:])
                    nc.vector.tensor_tensor(
                        o_g[:, i * E : (i + 1) * E],
                        ev[:],
                        pe_g[:, i * E : (i + 1) * E],
                        Alu.add,
                    )
            nc.sync.dma_start(out_d[st], o_g[:])

        load_tables(0)
        alloc_dmats(0)
        build_d(0)
        for k, (vt, g) in enumerate(stages):
            if g == 0 and vt + 1 < VT:
                load_tables(vt + 1)
            staged[(vt, g)] = emit_stage(k, vt, g)
            if g == 0 and vt + 1 < VT:
                alloc_dmats(vt + 1)
                build_d(vt + 1)
            if k >= 1:
                pvt, pg = stages[k - 1]
                emit_items(pvt, pg, staged.pop((pvt, pg)))
        pvt, pg = stages[-1]
        emit_items(pvt, pg, staged.pop((pvt, pg)))


    nc.finalize()
    return nc


_NC_CACHE: list = []


def _get_nc():
    if not _NC_CACHE:
        _NC_CACHE.append(build_bass())
    return _NC_CACHE[0]


def make_in_maps(sequence, flux_w, flux_b, time_w, time_b):
    import ml_dtypes

    sequence = np.asarray(sequence, dtype=np.float32)
    flux_w = np.asarray(flux_w, dtype=np.float32)
    time_w = np.asarray(time_w, dtype=np.float32)
    bsum = np.asarray(flux_b, dtype=np.float32) + np.asarray(time_b, dtype=np.float32)

    s1_all = sequence[:, :, 1]
    assert np.abs(s1_all).max() < S1_LIMIT, (
        f"positional channel exceeds direct-Sin range: {np.abs(s1_all).max():.3f} "
        f">= {S1_LIMIT:.3f}; raise KLO"
    )

    div = np.exp(
        np.arange(0, E, 2, dtype=np.float32) * np.float32(-math.log(10000.0) / E)
    ).astype(np.float32)
    # dv_full = [ div[KLO:] (336) | 48 lo sin lanes * 2/pi | 48 lo cos * 2/pi ]
    dv2p = (np.concatenate([div[:KLO], div[:KLO]]) * np.float32(2.0 / math.pi)).astype(
        np.float32
    )
    dv_ext = np.concatenate([div[KLO:], dv2p]).astype(np.float32)
    dv_rep = np.ascontiguousarray(np.broadcast_to(dv_ext, (128, KHI + 2 * KLO)))
    eye_bf = np.eye(128, dtype=np.float32).astype(ml_dtypes.bfloat16)

    # combo_n[b,v,h*KLO+k] = j - 4*rint((s1*d_k + j*pi/2)/2pi), j = h (0=sin,1=cos)
    jj = np.concatenate([np.zeros(KLO, np.float64), np.ones(KLO, np.float64)])
    dd = np.concatenate([div[:KLO], div[:KLO]]).astype(np.float64)
    ang = s1_all[:, :, None].astype(np.float64) * dd[None, None, :] + jj * (
        math.pi / 2.0
    )
    n = np.rint(ang / TWO_PI)
    combo_n = (jj[None, None, :] - 4.0 * n).astype(np.float32)
    assert np.abs(combo_n).max() <= 16, "combo codes exceed bf16-exact range"
    combo_bf = combo_n.astype(ml_dtypes.bfloat16)  # small ints: bf16-exact

    fw_bf = flux_w.astype(ml_dtypes.bfloat16)
    tw_bf = time_w.astype(ml_dtypes.bfloat16)
    bs_bf = bsum.astype(ml_dtypes.bfloat16)

    in_maps = []
    for c in range(N_CORES):
        v0, v1 = c * V_SHARD, (c + 1) * V_SHARD
        # [B, 512, 3] -> [128p, vt*B*3 + b*3 + ch]
        s = sequence[:, v0:v1, :].reshape(B, VT, 128, 3)
        seq_r = np.ascontiguousarray(s.transpose(2, 1, 0, 3)).reshape(128, VT * B * 3)
        # combo [B, 512, 2*KLO] -> [128p, (vt*B + b)*2*KLO + lane]
        cmb = combo_bf[:, v0:v1, :].reshape(B, VT, 128, 2 * KLO)
        cmb_r = np.ascontiguousarray(cmb.transpose(2, 1, 0, 3)).reshape(
            128, VT * B * 2 * KLO
        )
        in_maps.append(
            {
                "seq": seq_r,
                "fw": np.ascontiguousarray(fw_bf[v0:v1]),
                "tw": np.ascontiguousarray(tw_bf[v0:v1]),
                "bs": np.ascontiguousarray(bs_bf[v0:v1]),
                "dv": dv_rep,
                "combo": cmb_r,
                "eye": eye_bf,
            }
        )
    return in_maps


def run(in_maps, trace: bool = False):
    nc = _get_nc()
    return run_bass_kernel_spmd(nc, in_maps, list(range(N_CORES)), trace=trace)


def assemble(res) -> np.ndarray:
    """Reassemble per-core [NSTAGES, 128, GB*E] bf16 blocks into [B, V, E] f32."""
    cores = []
    for c in range(N_CORES):
        arr = np.asarray(res.results[c]["out"]).astype(np.float32)
        # [vt*2+g, p, i*E+e] -> [b, v, e] with b = g*GB+i, v = vt*128+p
        arr = arr.reshape(VT, B // GB, 128, GB, E)
        arr = arr.transpose(1, 3, 0, 2, 4).reshape(B, V_SHARD, E)
        cores.append(arr)
    return np.ascontiguousarray(np.concatenate(cores, axis=1))


def kernel(sequence, flux_w, flux_b, time_w, time_b) -> np.ndarray:
    in_maps = make_in_maps(sequence, flux_w, flux_b, time_w, time_b)
    res = run(in_maps)
    return assemble(res)



# revision 40
# speedup vs baseline: 1.1578x; 1.1578x over previous
"""Trainium2 Bass kernel for nn_BERTEmbedding (fused per-index affine + sinusoidal PE).

Math (per batch b, vocab-position v, embed index e):
    out[b,v,e] = s0[b,v]*flux_w[v,e] + flux_b[v,e]
               + s2[b,v]*time_w[v,e] + time_b[v,e]
               + (e even: sin(s1[b,v]*div[e/2]) ; e odd: cos(s1[b,v]*div[(e-1)/2]))

Sharding: vocab axis V=4096 split across 8 cores (512 rows each); every core
handles all 16 batches of its vocab shard.  Weight tables are sharded with the
vocab axis.

Device strategy (per core, 4 v-tiles x 16 batches = 64 work items of [128,768],
software-pipelined in 8 stages of GB=8 batches so every engine queue stays one
stage ahead of its consumers):
  - Tables fw/tw/bsum shipped bf16 (halves table DMA; bf16 matmuls are
    1 cycle/row at any moving width and halve LDWEIGHTS time).
  - TensorE: psum = diag(s0) @ fw + diag(s2) @ tw + I @ bsum, bf16 weights,
    f32 PSUM accumulate, 512/256 column splits per PSUM bank.
  - Diag tiles D[p, b*128+q] = eye[p,q] * s_ch[p,b]: batched GPSIMD broadcast
    chunks for vt0/vt2, per-item ScalarE Copy-with-scale for vt1/vt3 (plain 2D
    APs only - rearranged/squeezed scale APs corrupt ACT results).
  - ScalarE: pe tile via Sin activation per 8-batch group, laid out as
    [sin half | cos half].  ScalarE Sin valid on [-pi,pi]:
      k >= KLO: ang = s1*[dv_hi|dv2'] staged per-item on DVE (even v-tiles)
                or grouped GPSIMD broadcast mult (odd v-tiles);
                sin via bias=0, cos via bias=pi/2.
      k <  KLO: host ships integer phase codes combo_n (bf16-exact);
                r' = ang_lo + combo (one grouped GPSIMD add), then
                Sin(scale=pi/2).
  - VectorE: single merge out = psum + pe per item (interleaves sin/cos via
    read APs, doubles as the PSUM evacuation - DMA cannot read PSUM).
  - DMA: loads on the ACT HWDGE queue, stores on the SP queue (no
    head-of-line blocking between them); one 393KB store per work item.
NB (measured): GPSIMD execution stalls concurrent DVE ops, so GPSIMD work is
placed only where DVE has slack; perfetto slice durations include sem waits.
"""

import math

import numpy as np

try:
    import concourse.bass as bass
except ImportError:  # harness containers keep the repo at /opt/trn_rl_repo
    import sys

    sys.path.insert(0, "/opt/trn_rl_repo")
    import concourse.bass as bass

import concourse.bacc as bacc
import concourse.tile as tile
from concourse import mybir
from concourse.bass_utils import run_bass_kernel_spmd

B, V, E = 16, 4096, 768
EH = E // 2  # 384 angle lanes
KLO = 48  # angle lanes fixed up via the host combo tensor
KHI = EH - KLO  # 336 direct-sin lanes
# cos(ang_k) = 1.0 within ~1e-3 for k >= COSH (|ang| <= S1_LIMIT*10^(-k/96));
# those pe lanes are memset once instead of computed by ScalarE
COSH = 197
# sin(ang_k) = ang_k within ~1e-3 for k >= KSIN; staging writes ang into the
# pe tile directly and ScalarE only evaluates Sin on lanes [KLO, KSIN)
KSIN = 138
N_CORES = 8
V_SHARD = V // N_CORES  # 512
# merges per stage read psum directly on the DVE (1x); the rest go through a
# ScalarE psum->bf16 evacuation and an all-bf16 2x DVE add
N_DIRECT = 8
VT = V_SHARD // 128  # 4 v-tiles per core
GB = 8  # batches per pe/sin group
F32 = mybir.dt.float32
BF16 = mybir.dt.bfloat16

TWO_PI = 2.0 * math.pi
HALF_PI = float(np.float32(math.pi / 2.0))
# keep reduced angles strictly inside ScalarE's [-pi, pi] spline domain
SIN_SAFETY = 1.0 - 1e-6
# direct-Sin lanes need |s1|*d_KLO + pi/2 <= pi
S1_LIMIT = (math.pi / 2.0) / math.exp(-KLO * math.log(10000.0) / EH)



def build_bass() -> "bass.Bass":
    from contextlib import ExitStack

    nc = bacc.Bacc(
        "TRN2",
        target_bir_lowering=False,
        debug=False,
        num_devices=N_CORES,
    )
    Alu = mybir.AluOpType

    # dv_full = [ div[KLO:EH) (336) | dv2' (96, scaled 2/pi) ]
    KX = KHI + 2 * KLO  # 432
    seq_d = nc.dram_tensor("seq", [128, VT * B * 3], F32, kind="ExternalInput")
    fw_d = nc.dram_tensor("fw", [V_SHARD, E], BF16, kind="ExternalInput")
    tw_d = nc.dram_tensor("tw", [V_SHARD, E], BF16, kind="ExternalInput")
    bs_d = nc.dram_tensor("bs", [V_SHARD, E], BF16, kind="ExternalInput")
    dv_d = nc.dram_tensor("dv", [128, KX], F32, kind="ExternalInput")
    cmb_d = nc.dram_tensor("combo", [128, VT * B * 2 * KLO], BF16, kind="ExternalInput")
    eye_d = nc.dram_tensor("eye", [128, 128], BF16, kind="ExternalInput")
    # bf16 output, one [128, GB*E] block per pipeline stage; host reassembles
    NSTAGES = VT * (B // GB)
    out_d = nc.dram_tensor("out", [NSTAGES, 128, GB * E], BF16, kind="ExternalOutput")

    with tile.TileContext(nc) as tc, ExitStack() as ctx:
        const_pool = ctx.enter_context(tc.tile_pool(name="const", bufs=1))
        tab_pool = ctx.enter_context(tc.tile_pool(name="tables", bufs=2))
        dmat_pool = ctx.enter_context(tc.tile_pool(name="dmat", bufs=4))
        ang_pool = ctx.enter_context(tc.tile_pool(name="ang", bufs=4))
        ev_pool = ctx.enter_context(tc.tile_pool(name="ev", bufs=6))
        out_pool = ctx.enter_context(tc.tile_pool(name="out", bufs=4))
        psum_pool = ctx.enter_context(tc.tile_pool(name="psum", bufs=4, space="PSUM"))

        zero_t = const_pool.tile([128, 1], F32, tag="zero")
        nc.vector.memset(zero_t[:], 0.0)
        hpi_t = const_pool.tile([128, 1], F32, tag="hpi")
        nc.vector.memset(hpi_t[:], HALF_PI)

        seq_t = const_pool.tile([128, VT * B * 3], F32, tag="seq")
        nc.sync.dma_start(seq_t[:], seq_d[:])
        dv_t = const_pool.tile([128, KX], F32, tag="dv")
        nc.sync.dma_start(dv_t[:], dv_d[:])
        eye_t = const_pool.tile([128, 128], BF16, tag="eye")
        nc.sync.dma_start(eye_t[:], eye_d[:])

        # ping-pong pe tiles in the NATIVE interleaved layout (even col 2k =
        # sin lane k, odd col 2k+1 = cos lane k) so merges read flat APs.
        # cos lanes k >= COSH hold constant 1.0 (memset once, never
        # overwritten: ScalarE only writes sin + cos-head lanes)
        pe_tiles = []
        for pp in range(3):
            pe_t = const_pool.tile([128, GB * E], BF16, tag=f"pe{pp}")
            nc.vector.memset(
                pe_t[:].rearrange("p (i e) -> p i e", i=GB)[
                    :, :, 2 * COSH + 1 : E : 2
                ],
                1.0,
            )
            pe_tiles.append(pe_t)

        item_idx = 0
        # ---- software-pipelined emission over 8 stages (vt, g) ----
        # Stage k's production (angles, r4, sins, next-vtile diag builds) is
        # emitted BEFORE stage k-1's consumption (matmuls, merges, stores) so
        # no engine queue head-of-line blocks on a consumer stall.
        stages = [(vt, g) for vt in range(VT) for g in range(B // GB)]
        tabs: dict = {}
        dmats: dict = {}
        staged: dict = {}

        def seq_view(vt):
            return seq_t[:, vt * B * 3 : (vt + 1) * B * 3].rearrange(
                "p (b c) -> p b c", c=3
            )

        def load_tables(vt):
            fw_t = tab_pool.tile([128, E], BF16, tag="fw")
            nc.sync.dma_start(fw_t[:], fw_d[vt * 128 : (vt + 1) * 128, :])
            tw_t = tab_pool.tile([128, E], BF16, tag="tw")
            nc.sync.dma_start(tw_t[:], tw_d[vt * 128 : (vt + 1) * 128, :])
            bs_t = tab_pool.tile([128, E], BF16, tag="bs")
            nc.sync.dma_start(bs_t[:], bs_d[vt * 128 : (vt + 1) * 128, :])
            cmb_t = tab_pool.tile([128, B * 2 * KLO], BF16, tag="cmb")
            nc.sync.dma_start(
                cmb_t[:], cmb_d[:, vt * B * 2 * KLO : (vt + 1) * B * 2 * KLO]
            )
            tabs[vt] = (fw_t, tw_t, bs_t, cmb_t)

        def alloc_dmats(vt):
            dmats[vt] = {}
            for ch in (0, 2):
                d_t = dmat_pool.tile([128, B * 128], BF16, tag=f"d{ch}")
                dmats[vt][ch] = d_t

        def build_d(vt):
            # diag build D[p, b*128+q] = eye[p,q] * s_ch[p,b]:
            # batched broadcast chunks on GPSIMD for every v-tile
            sv = seq_view(vt)
            for ch in (0, 2):
                d3 = dmats[vt][ch][:].rearrange("p (b q) -> p b q", b=B)
                for h0 in (0, B // 2):
                    eye_b = eye_t[:].unsqueeze(1).broadcast_to((128, B // 2, 128))
                    s_b = sv[:, h0 : h0 + B // 2, ch : ch + 1].broadcast_to(
                        (128, B // 2, 128)
                    )
                    nc.gpsimd.tensor_tensor(
                        d3[:, h0 : h0 + B // 2, :], eye_b, s_b, Alu.mult
                    )

        def emit_stage(k, vt, g):
            g0, g1 = g * GB, (g + 1) * GB
            ang_g = ang_pool.tile([128, GB * 2 * KLO], F32, tag="ang")
            r4_g = ang_pool.tile([128, GB * 2 * KLO], F32, tag="r4")
            cmb_t = tabs[vt][3]
            pe_g = pe_tiles[k % 3]
            pe3 = pe_g[:].rearrange("p (i e) -> p i e", i=GB)
            sv = seq_view(vt)
            s1b = sv[:, g0:g1, 1:2]  # [128, GB, 1] positional channel
            # staging op2': ang_k for k >= KLO straight into the pe sin
            # lanes (sin(x)=x holds for k >= KSIN; lanes [KLO,KSIN) are
            # Sin-ed in place below; lanes [KSIN,COSH) feed cos first).
            # Per-item on ScalarE: the DVE queue (merges) is the pacer.
            for i, b in enumerate(range(g0, g1)):
                col = vt * B * 3 + b * 3 + 1
                nc.scalar.mul(
                    pe_g[:, i * E + 2 * KLO : (i + 1) * E : 2],
                    dv_t[:, 0:KHI],
                    seq_t[:, col : col + 1],
                )
            # grouped staging op1': lo r0 lanes (dv2' scaled 2/pi) — on the
            # DVE: GPSIMD runs these ~3x slower and its SBUF-port traffic
            # stalls concurrent DVE work
            nc.vector.tensor_tensor(
                ang_g[:].rearrange("p (i l) -> p i l", i=GB),
                dv_t[:, KHI:KX].unsqueeze(1).broadcast_to((128, GB, 2 * KLO)),
                s1b.broadcast_to((128, GB, 2 * KLO)),
                Alu.mult,
            )
            # lo codes r' = r0 + combo, one grouped add
            nc.gpsimd.tensor_tensor(
                r4_g[:].rearrange("p (i l) -> p i l", i=GB),
                ang_g[:].rearrange("p (i l) -> p i l", i=GB),
                cmb_t[:, g * GB * 2 * KLO : (g + 1) * GB * 2 * KLO].rearrange(
                    "p (i l) -> p i l", i=GB
                ),
                Alu.add,
            )
            # cos head: reads the staged angles from the sin lanes (before
            # the in-place Sin consumes them), writes the odd cos columns
            nc.scalar.activation(
                pe3[:, :, 2 * KLO + 1 : 2 * COSH : 2],
                pe3[:, :, 2 * KLO : 2 * COSH : 2],
                mybir.ActivationFunctionType.Sin,
                bias=hpi_t[:],
                scale=1.0,
            )
            # sin head, in place over the staged angles
            nc.scalar.activation(
                pe3[:, :, 2 * KLO : 2 * KSIN : 2],
                pe3[:, :, 2 * KLO : 2 * KSIN : 2],
                mybir.ActivationFunctionType.Sin,
                bias=zero_t[:],
                scale=1.0,
            )
            # lo block: r4 layout per item is [48 sin | 48 cos]
            nc.scalar.activation(
                pe3[:, :, 0 : 2 * KLO].rearrange("p i (q h) -> p i h q", h=2),
                r4_g[:].rearrange("p (i h q) -> p i h q", i=GB, h=2),
                mybir.ActivationFunctionType.Sin,
                bias=zero_t[:],
                scale=HALF_PI * SIN_SAFETY,
            )
            return pe_g

        def emit_items(vt, g, pe_g):
            fw_t, tw_t, bs_t, _ = tabs[vt]
            st = vt * (B // GB) + g
            # per-stage bf16 output block, one batched DMA store per stage
            o_g = out_pool.tile([128, GB * E], BF16, tag="o")
            for i, b in enumerate(range(g * GB, (g + 1) * GB)):
                # psum = diag(s0)@fw + diag(s2)@tw + I@bsum, split 512/256
                # to keep each matmul inside one PSUM bank
                # stationary-major order so consecutive matmuls share one
                # LDWEIGHTS where walrus can elide the reload
                ps = psum_pool.tile([128, E], F32, tag="ps")
                for st_t, mov_t, first, last in (
                    (dmats[vt][0][:, b * 128 : (b + 1) * 128], fw_t, True, False),
                    (dmats[vt][2][:, b * 128 : (b + 1) * 128], tw_t, False, False),
                    (eye_t[:], bs_t, False, True),
                ):
                    for lo, hi in ((0, 512), (512, E)):
                        nc.tensor.matmul(
                            ps[:, lo:hi],
                            st_t,
                            mov_t[:, lo:hi],
                            start=first,
                            stop=last,
                        )
                # merge (flat APs: pe is natively interleaved).  Most items
                # go via a ScalarE psum->bf16 evacuation so the DVE add runs
                # in all-bf16 2x_1p mode; the rest read psum directly at 1x.
                if i < N_DIRECT:
                    nc.vector.tensor_tensor(
                        o_g[:, i * E : (i + 1) * E],
                        ps[:],
                        pe_g[:, i * E : (i + 1) * E],
                        Alu.add,
                    )
                else:
                    ev = ev_pool.tile([128, E], BF16, tag="ev")
                    nc.scalar.copy(ev[:], ps[:])
                    nc.vector.tensor_tensor(
                        o_g[:, i * E : (i + 1) * E],
                        ev[:],
                        pe_g[:, i * E : (i + 1) * E],
                        Alu.add,
                    )
            nc.sync.dma_start(out_d[st], o_g[:])

        load_tables(0)
        alloc_dmats(0)
        build_d(0)
        for k, (vt, g) in enumerate(stages):
            if g == 0 and vt + 1 < VT:
                load_tables(vt + 1)
            staged[(vt, g)] = emit_stage(k, vt, g)
            if g == 0 and vt + 1 < VT:
                alloc_dmats(vt + 1)
                build_d(vt + 1)
            if k >= 1:
                pvt, pg = stages[k - 1]
                emit_items(pvt, pg, staged.pop((pvt, pg)))
        pvt, pg = stages[-1]
        emit_items(pvt, pg, staged.pop((pvt, pg)))


    nc.finalize()
    return nc


_NC_CACHE: list = []


def _get_nc():
    if not _NC_CACHE:
        _NC_CACHE.append(build_bass())
    return _NC_CACHE[0]


def make_in_maps(sequence, flux_w, flux_b, time_w, time_b):
    import ml_dtypes

    sequence = np.asarray(sequence, dtype=np.float32)
    flux_w = np.asarray(flux_w, dtype=np.float32)
    time_w = np.asarray(time_w, dtype=np.float32)
    bsum = np.asarray(flux_b, dtype=np.float32) + np.asarray(time_b, dtype=np.float32)

    s1_all = sequence[:, :, 1]
    assert np.abs(s1_all).max() < S1_LIMIT, (
        f"positional channel exceeds direct-Sin range: {np.abs(s1_all).max():.3f} "
        f">= {S1_LIMIT:.3f}; raise KLO"
    )

    div = np.exp(
        np.arange(0, E, 2, dtype=np.float32) * np.float32(-math.log(10000.0) / E)
    ).astype(np.float32)
    # dv_full = [ div[KLO:] (336) | 48 lo sin lanes * 2/pi | 48 lo cos * 2/pi ]
    dv2p = (np.concatenate([div[:KLO], div[:KLO]]) * np.float32(2.0 / math.pi)).astype(
        np.float32
    )
    dv_ext = np.concatenate([div[KLO:], dv2p]).astype(np.float32)
    dv_rep = np.ascontiguousarray(np.broadcast_to(dv_ext, (128, KHI + 2 * KLO)))
    eye_bf = np.eye(128, dtype=np.float32).astype(ml_dtypes.bfloat16)

    # combo_n[b,v,h*KLO+k] = j - 4*rint((s1*d_k + j*pi/2)/2pi), j = h (0=sin,1=cos)
    jj = np.concatenate([np.zeros(KLO, np.float64), np.ones(KLO, np.float64)])
    dd = np.concatenate([div[:KLO], div[:KLO]]).astype(np.float64)
    ang = s1_all[:, :, None].astype(np.float64) * dd[None, None, :] + jj * (
        math.pi / 2.0
    )
    n = np.rint(ang / TWO_PI)
    combo_n = (jj[None, None, :] - 4.0 * n).astype(np.float32)
    assert np.abs(combo_n).max() <= 16, "combo codes exceed bf16-exact range"
    combo_bf = combo_n.astype(ml_dtypes.bfloat16)  # small ints: bf16-exact

    fw_bf = flux_w.astype(ml_dtypes.bfloat16)
    tw_bf = time_w.astype(ml_dtypes.bfloat16)
    bs_bf = bsum.astype(ml_dtypes.bfloat16)

    in_maps = []
    for c in range(N_CORES):
        v0, v1 = c * V_SHARD, (c + 1) * V_SHARD
        # [B, 512, 3] -> [128p, vt*B*3 + b*3 + ch]
        s = sequence[:, v0:v1, :].reshape(B, VT, 128, 3)
        seq_r = np.ascontiguousarray(s.transpose(2, 1, 0, 3)).reshape(128, VT * B * 3)
        # combo [B, 512, 2*KLO] -> [128p, (vt*B + b)*2*KLO + lane]
        cmb = combo_bf[:, v0:v1, :].reshape(B, VT, 128, 2 * KLO)
        cmb_r = np.ascontiguousarray(cmb.transpose(2, 1, 0, 3)).reshape(
            128, VT * B * 2 * KLO
        )
        in_maps.append(
            {
                "seq": seq_r,
                "fw": np.ascontiguousarray(fw_bf[v0:v1]),
                "tw": np.ascontiguousarray(tw_bf[v0:v1]),
                "bs": np.ascontiguousarray(bs_bf[v0:v1]),
                "dv": dv_rep,
                "combo": cmb_r,
                "eye": eye_bf,
            }
        )
    return in_maps


def run(in_maps, trace: bool = False):
    nc = _get_nc()
    return run_bass_kernel_spmd(nc, in_maps, list(range(N_CORES)), trace=trace)


def assemble(res) -> np.ndarray:
    """Reassemble per-core [NSTAGES, 128, GB*E] bf16 blocks into [B, V, E] f32."""
    cores = []
    for c in range(N_CORES):
        arr = np.asarray(res.results[c]["out"]).astype(np.float32)
        # [vt*2+g, p, i*E+e] -> [b, v, e] with b = g*GB+i, v = vt*128+p
        arr = arr.reshape(VT, B // GB, 128, GB, E)
        arr = arr.transpose(1, 3, 0, 2, 4).reshape(B, V_SHARD, E)
        cores.append(arr)
    return np.ascontiguousarray(np.concatenate(cores, axis=1))


def kernel(sequence, flux_w, flux_b, time_w, time_b) -> np.ndarray:
    in_maps = make_in_maps(sequence, flux_w, flux_b, time_w, time_b)
    res = run(in_maps)
    return assemble(res)

